# revision 59
# baseline (speedup 1.0000x reference)
"""Trainium2 Bass kernel for nn_BipartitePooling (GATv2 bipartite pooling).

Sharding: one graph per NeuronCore (8 graphs, 8 cores, no collectives).
Per core, for its graph's nodes (padded to N_CAP columns, pad columns pushed
to -inf logits via a mask row):

  h_srcT[f,n] = W_l^T @ xT                                   (PE)
  logit[4r+h, n] = 0.8*A^T relu(h_srcT + srT[:,r])           (relu stream on
      + (0.2*W_l@A)^T @ xT + Q[r,h] + maskbias[n]             ACT+DVE, reduce
                                                              on PE into PSUM)
  p = exp(logit)               (ACT, straight from PSUM; max-shift skipped —
                                logits are O(+-10) so fp32 exp is safe)
  G[m,:], s[m] = p @ [x | 1]   (PE, via PE-transposed p tiles)
  out = (G/s) @ W_l + bias     (tiny [128,128] tail)

n is processed in two column-halves; half 0's exp/transpose/aggregate tail is
folded into half 1's producer-bound PE stream. The per-seed channel reduce
uses 32-column zero-padded stationaries cycling the four PE column-groups
(consecutive matmuls hit different col_grps and overlap ~2.6x). Seed-side
prep (srT = seed@W_r, fused 0.2*W_l@A, Q, masks) is host weight prep.
"""
import os
import sys
from contextlib import ExitStack

import numpy as np

for _p in ("/root/.axon_site", "/root/.axon_site/_ro/trn_rl_repo",
           "/root/.axon_site/_ro/pypackages", "/opt/trn_rl_repo"):
    if os.path.isdir(_p) and _p not in sys.path:
        sys.path.append(_p)

import concourse.bass as bass
import concourse.bacc as bacc
import concourse.tile as tile
import concourse.mybir as mybir
from concourse.bass_utils import run_bass_kernel_spmd

F32 = mybir.dt.float32
BF16 = mybir.dt.bfloat16
AF = mybir.ActivationFunctionType
ALU = mybir.AluOpType
NP_BF16 = mybir.dt.np(BF16)

B, F, RATIO, H, C = 8, 128, 32, 4, 32
N_CAP = 2304


def _halves_of(n_cap):
    # sections of <=1024 columns: the per-section logit PSUM tile is exactly
    # <=2 banks, letting two sections double-buffer within the 8-bank budget
    out = []
    o = 0
    while o < n_cap:
        sz = min(1024, n_cap - o)
        out.append((o, sz))
        o += sz
    return out

# traversal order interleaves the four 32-row PSUM column-groups (r//8) so
# consecutive PE matmuls target different col_grps of the array
R_ORDER = [8 * (i % 4) + i // 4 for i in range(RATIO)]
# relu-stream engine by traversal position so ACT/DVE work interleaves
# (GPSIMD tensor ops measured 17us/op + SBUF port contention — never use)
ENGINE_OF_POS = ["A" if i % 3 == 1 else "V" for i in range(RATIO)]
# seeds handled by DVE use the one-op form max(h, -s) = relu(h+s) - s; the
# missing +0.8*A^T s term is folded into the host Q correction per row
DVE_SEEDS = {r for i, r in enumerate(R_ORDER) if ENGINE_OF_POS[i] == "V"}

_CACHE = {}


def _chunks_of(off0, size):
    out = []
    o = 0
    while o < size:
        sz = min(512, size - o)
        out.append((off0 + o, sz))
        o += sz
    return out


def build_bass(n_cap=N_CAP):
    nc = bacc.Bacc("TRN2", target_bir_lowering=False, debug=False)
    nt = n_cap // 128
    xT = nc.declare_dram_parameter("xT", [128, n_cap], BF16, isOutput=False)
    xn = nc.declare_dram_parameter("xn", [128, nt * 129], BF16, isOutput=False)
    wl = nc.declare_dram_parameter("W_l", [128, 128], BF16, isOutput=False)
    was = nc.declare_dram_parameter("WAS", [128, 128], BF16, isOutput=False)
    srT = nc.declare_dram_parameter("srTn", [128, 64], F32, isOutput=False)
    abig = nc.declare_dram_parameter("A_big", [128, RATIO * 32], BF16,
                                     isOutput=False)
    selqm = nc.declare_dram_parameter("SelQm", [2, 128], BF16, isOutput=False)
    mask = nc.declare_dram_parameter("maskones", [2, n_cap], BF16, isOutput=False)
    bias1 = nc.declare_dram_parameter("bias1", [1, 128], F32, isOutput=False)
    qcol = nc.declare_dram_parameter("qcol", [128, 1], F32, isOutput=False)
    ident = nc.declare_dram_parameter("ident", [128, 128], BF16, isOutput=False)
    outp = nc.declare_dram_parameter("out", [128, 128], F32, isOutput=True)

    halves = _halves_of(n_cap)

    with ExitStack() as ctx:
        tc = ctx.enter_context(tile.TileContext(nc))
        consts = ctx.enter_context(tc.tile_pool(name="consts", bufs=1))
        big = ctx.enter_context(tc.tile_pool(name="big", bufs=1))
        rpool = ctx.enter_context(tc.tile_pool(name="rpool", bufs=12))
        ps_tmp = ctx.enter_context(tc.tile_pool(name="ps_tmp", bufs=2, space="PSUM"))
        ps_lg = ctx.enter_context(tc.tile_pool(name="ps_lg", bufs=2, space="PSUM"))
        ps_ptr = ctx.enter_context(tc.tile_pool(name="ps_ptr", bufs=1, space="PSUM"))
        ps_os = ctx.enter_context(tc.tile_pool(name="ps_os", bufs=1, space="PSUM"))

        # ---- inputs in, spread across the sync/gpsimd DMA queues (keep the
        #      ACT queue free for the relu stream) ----
        wl_sb = consts.tile([128, 128], BF16, tag="wl")
        nc.sync.dma_start(out=wl_sb, in_=wl[:])
        xT_sb = big.tile([128, n_cap], BF16, tag="xT")
        was_sb = consts.tile([128, 128], BF16, tag="was")
        nc.scalar.dma_start(out=was_sb, in_=was[:])
        xq = [0] + [(n_cap * i // 512) * 128 for i in (1, 2, 3)] + [n_cap]
        for qi, (a, b) in enumerate(zip(xq, xq[1:])):
            eng = nc.sync if qi % 2 == 0 else nc.scalar
            eng.dma_start(out=xT_sb[:, a:b], in_=xT[:, a:b])
        xn_sb = big.tile([128, nt * 129], BF16, tag="xn")
        nc.scalar.dma_start(out=xn_sb, in_=xn[:])
        abig_sb = consts.tile([128, RATIO * 32], BF16, tag="abig")
        nc.sync.dma_start(out=abig_sb, in_=abig[:])
        ident_sb = consts.tile([128, 128], BF16, tag="ident")
        nc.sync.dma_start(out=ident_sb, in_=ident[:])
        selqm_sb = consts.tile([2, 128], BF16, tag="selqm")
        nc.gpsimd.dma_start(out=selqm_sb, in_=selqm[:])
        mask_sb = consts.tile([2, n_cap], BF16, tag="mask")
        nc.gpsimd.dma_start(out=mask_sb, in_=mask[:])
        srT_sb = consts.tile([128, 64], F32, tag="srt")
        nc.gpsimd.dma_start(out=srT_sb, in_=srT[:])
        qcol_sb = consts.tile([128, 1], F32, tag="qcol")
        nc.gpsimd.dma_start(out=qcol_sb, in_=qcol[:])
        biasb_sb = consts.tile([128, 128], F32, tag="biasb")
        b1 = bias1[0:1, :]
        nc.gpsimd.dma_start(
            out=biasb_sb,
            in_=bass.AP(tensor=b1.tensor, offset=b1.offset, ap=[[0, 128], [1, 128]]),
        )

        # ---- h_srcT = W_l^T @ xT (ACT relus read the full width, so all
        #      chunks are evacuated up front) ----
        hsrcT_sb = big.tile([128, n_cap], BF16, tag="hsrcT")
        for off, sz in _chunks_of(0, n_cap):
            pt = ps_tmp.tile([128, 512], F32, tag="tmp")
            nc.tensor.matmul(pt[:, :sz], lhsT=wl_sb, rhs=xT_sb[:, off:off + sz],
                             start=True, stop=True)
            nc.vector.tensor_copy(hsrcT_sb[:, off:off + sz], pt[:, :sz])
        rapool = ctx.enter_context(tc.tile_pool(name="rapool", bufs=1))
        p_sb = big.tile([128, n_cap], BF16, tag="p")
        pT_sb = big.tile([128, nt * 128], BF16, tag="pT")
        g_ps = ps_os.tile([128, 129], F32, tag="G")

        def emit_exp(h_off, h_sz, lg):
            # softmax numerator straight out of PSUM (no logit evacuation)
            nc.scalar.activation(p_sb[:, h_off:h_off + h_sz],
                                 lg[:, 0:h_sz], AF.Exp, bias=qcol_sb[:, 0:1])

        def emit_transpose(tiles):
            # PE-transpose p tiles (4 per PSUM bank) and evacuate
            for b0 in range(0, len(tiles), 4):
                bt = tiles[b0:b0 + 4]
                ptr = ps_ptr.tile([128, 512], BF16, tag="ptr")
                for j, t in enumerate(bt):
                    nc.tensor.transpose(ptr[:, 128 * j:128 * (j + 1)],
                                        p_sb[:, 128 * t:128 * (t + 1)], ident_sb)
                nc.vector.tensor_copy(
                    pT_sb[:, 128 * bt[0]:128 * (bt[-1] + 1)],
                    ptr[:, :128 * len(bt)])

        def emit_agg(tiles, first, last):
            for t in tiles:
                nc.tensor.matmul(g_ps[:, :],
                                 lhsT=pT_sb[:, 128 * t:128 * (t + 1)],
                                 rhs=xn_sb[:, 129 * t:129 * (t + 1)],
                                 start=(first and t == tiles[0]),
                                 stop=(last and t == tiles[-1]))

        pending = None
        ract = {}
        for hi, (h_off, h_sz) in enumerate(halves):
            chunks = _chunks_of(h_off, h_sz)
            # per-section logit PSUM tile (<=2 banks, double-buffered pool);
            # each matmul stays within a single 2KB bank slice of it
            lg = ps_lg.tile([128, 1024], F32, tag="lg")
            for ci, (off, sz) in enumerate(chunks):
                o = off - h_off
                nc.tensor.matmul(lg[:, o:o + sz], lhsT=was_sb,
                                 rhs=xT_sb[:, off:off + sz],
                                 start=True, stop=False, skip_group_check=True)
                nc.tensor.matmul(lg[:, o:o + sz], lhsT=selqm_sb,
                                 rhs=mask_sb[:, off:off + sz],
                                 start=False, stop=False, skip_group_check=True)

            for pos, r in enumerate(R_ORDER):
                if ENGINE_OF_POS[pos] == "A":
                    # ACT seeds compute the full width once (first section)
                    # and later sections reuse the tile
                    if r not in ract:
                        RA = rapool.tile([128, n_cap], BF16, tag=f"RA{r}")
                        nc.scalar.activation(RA, hsrcT_sb, AF.Relu,
                                             bias=srT_sb[:, r:r + 1], scale=1.0)
                        ract[r] = RA
                    R, base = ract[r], 0
                else:
                    R = rpool.tile([128, h_sz], BF16, tag=f"R{hi}")
                    nc.vector.tensor_scalar_max(
                        R, hsrcT_sb[:, h_off:h_off + h_sz],
                        srT_sb[:, 32 + r:33 + r])
                    base = h_off
                g32 = 32 * (r // 8)
                for ci, (off, sz) in enumerate(chunks):
                    o = off - h_off
                    nc.tensor.matmul(lg[g32:g32 + 32, o:o + sz],
                                     lhsT=abig_sb[:, 32 * r:32 * r + 32],
                                     rhs=R[:, off - base:off - base + sz],
                                     start=False, stop=(r % 8 == 7),
                                     tile_position=(0, g32),
                                     skip_group_check=True)
                if pos == 8 and pending is not None:
                    # fold the previous section's transposes into this
                    # section's (producer-bound) PE stream ...
                    emit_transpose(pending[0])
                if pos == 18 and pending is not None:
                    # ... and its aggregation matmuls a bit later, giving the
                    # DVE time to evacuate the transposed tiles
                    emit_agg(*pending)
                    pending = None

            if hi < len(halves) - 1:
                emit_exp(h_off, h_sz, lg)
                pending = (list(range(h_off // 128, (h_off + h_sz) // 128)),
                           hi == 0, False)
            else:
                # final section: pipeline exp -> transpose -> aggregate
                for off, sz in _chunks_of(0, h_sz):
                    nc.scalar.activation(
                        p_sb[:, h_off + off:h_off + off + sz],
                        lg[:, off:off + sz], AF.Exp, bias=qcol_sb[:, 0:1])
                    tiles = list(range((h_off + off) // 128,
                                       (h_off + off + sz) // 128))
                    emit_transpose(tiles)
                    emit_agg(tiles, first=(hi == 0 and off == 0),
                             last=(off + sz == h_sz))

        # ---- tail: out = (G/s) @ W_l + bias ----
        srecip = big.tile([128, 1], F32, tag="srecip")
        nc.vector.reciprocal(srecip, g_ps[:, 128:129])
        gn_sb = big.tile([128, 128], BF16, tag="gn")
        nc.vector.tensor_scalar(gn_sb, g_ps[:, 0:128], scalar1=srecip,
                                scalar2=None, op0=ALU.mult)
        gnT_ps = ps_tmp.tile([128, 128], BF16, tag="tmp")
        nc.tensor.transpose(gnT_ps, gn_sb, ident_sb)
        gnT_sb = big.tile([128, 128], BF16, tag="gnT")
        nc.vector.tensor_copy(gnT_sb, gnT_ps)
        out_ps = ps_tmp.tile([128, 128], F32, tag="tmp")
        nc.tensor.matmul(out_ps, lhsT=gnT_sb, rhs=wl_sb, start=True, stop=True)
        out_sb = big.tile([128, 128], F32, tag="out_sb")
        nc.vector.tensor_add(out_sb, out_ps, biasb_sb)

        # full [m=4r+h, f] result out; host extracts out[r, 32h:32h+32] =
        # out_sb[4r+h, 32h:32h+32] during unshard
        nc.sync.dma_start(out=outp[:], in_=out_sb[:])

    nc.compile()
    return nc


def host_prep(x, batch, seed_nodes, W_l, W_r, att, bias, n_cap=N_CAP):
    f32 = np.float32
    x = np.asarray(x, f32)
    batch = np.asarray(batch).astype(np.int32)
    seed_nodes = np.asarray(seed_nodes, f32)
    W_l = np.asarray(W_l, f32)
    W_r = np.asarray(W_r, f32)
    att = np.asarray(att, f32)
    bias = np.asarray(bias, f32)
    nt = n_cap // 128

    order = np.argsort(batch, kind="stable")
    x_sorted = x[order]
    counts = np.bincount(batch[order], minlength=B)
    offs = np.concatenate([[0], np.cumsum(counts)])

    seed_hr = seed_nodes @ W_r                       # [32,128]
    A = np.zeros((F, H), f32)
    for h in range(H):
        A[h * C:(h + 1) * C, h] = att[h]
    # A_big[:, 32r + 4(r%8) + h] = 0.8*A[:, h]; the per-seed matmul uses the
    # 32-col slice [:, 32r:32r+32] writing PSUM rows [32*(r//8), +32) so
    # row m = 4r+h while output base partitions stay 32-aligned.
    A_big = np.zeros((F, RATIO * 32), f32)
    for r in range(RATIO):
        A_big[:, 32 * r + 4 * (r % 8):32 * r + 4 * (r % 8) + 4] = 0.8 * A
    m = np.arange(128)
    WAS = 0.2 * (W_l @ A)[:, m % 4]                  # [128, 128]
    # per-row constant: 0.2*A^T s_r always, plus 0.8*A^T s_r for seeds whose
    # relu runs as max(h, -s) on DVE (their A-matmul contribution is missing
    # the +0.8*A^T s term)
    AS = np.einsum("rf,fh->hr", seed_hr, A)          # [4,32]
    qscale = np.array([1.0 if (mm // 4) in DVE_SEEDS else 0.2
                       for mm in m], f32)
    qcol = np.ascontiguousarray((AS[m % 4, m // 4] * qscale)[:, None])
    SelQm = np.zeros((2, 128), f32)
    SelQm[1, :] = 1.0

    shared = dict(
        W_l=np.ascontiguousarray(W_l.astype(NP_BF16)),
        WAS=np.ascontiguousarray(WAS.astype(NP_BF16)),
        srTn=np.ascontiguousarray(
            np.concatenate([seed_hr.T, -seed_hr.T], axis=1)),
        A_big=A_big.astype(NP_BF16),
        SelQm=SelQm.astype(NP_BF16),
        bias1=np.ascontiguousarray(bias[None, :]),
        qcol=qcol,
        ident=np.eye(128, dtype=f32).astype(NP_BF16),
    )
    in_maps = []
    for b in range(B):
        n_b = int(counts[b])
        assert n_b <= n_cap, f"graph {b}: {n_b} nodes > N_CAP {n_cap}"
        xb = np.zeros((n_cap, F), f32)
        xb[:n_b] = x_sorted[offs[b]:offs[b + 1]]
        # xn: [x | 1] tiled into the SBUF layout [128, nt*129]
        x1 = np.concatenate([xb, np.ones((n_cap, 1), f32)], axis=1)
        xn = np.ascontiguousarray(
            x1.reshape(nt, 128, 129).transpose(1, 0, 2).reshape(128, nt * 129))
        maskones = np.zeros((2, n_cap), f32)
        maskones[0, :] = 1.0            # multiplies SelQm row 0 (Q correction)
        maskones[1, n_b:] = -50.0       # multiplies SelQm row 1 (ones)
        in_maps.append(dict(
            shared,
            xT=np.ascontiguousarray(xb.T.astype(NP_BF16)),
            xn=xn.astype(NP_BF16),
            maskones=maskones.astype(NP_BF16),
        ))
    return in_maps


def kernel(x, batch, seed_nodes, W_l, W_r, att, bias):
    counts = np.bincount(np.asarray(batch).astype(np.int64), minlength=B)
    n_cap = max(2048, int(-(-int(counts.max()) // 128) * 128))
    if n_cap not in _CACHE:
        _CACHE[n_cap] = build_bass(n_cap)
    nc = _CACHE[n_cap]
    in_maps = host_prep(x, batch, seed_nodes, W_l, W_r, att, bias, n_cap=n_cap)
    res = run_bass_kernel_spmd(nc, in_maps, core_ids=list(range(B)))
    out = np.concatenate([unshard_core(np.asarray(res.results[i]["out"]))
                          for i in range(B)], axis=0)
    new_batch = np.repeat(np.arange(B, dtype=np.int32), RATIO)
    return out, new_batch


def unshard_core(out128):
    out = np.empty((RATIO, F), np.float32)
    for h in range(H):
        out[:, 32 * h:32 * (h + 1)] = out128[h::4, 32 * h:32 * (h + 1)]
    return out


# revision 60
# speedup vs baseline: 1.0985x; 1.0985x over previous
"""Trainium2 Bass kernel for nn_BipartitePooling (GATv2 bipartite pooling).

Sharding: one graph per NeuronCore (8 graphs, 8 cores, no collectives).
Per core, for its graph's nodes (padded to N_CAP columns, pad columns pushed
to -inf logits via a mask row):

  h_srcT[f,n] = W_l^T @ xT                                   (PE)
  logit[4r+h, n] = 0.8*A^T relu(h_srcT + srT[:,r])           (relu stream on
      + (0.2*W_l@A)^T @ xT + Q[r,h] + maskbias[n]             ACT+DVE, reduce
                                                              on PE into PSUM)
  p = exp(logit)               (ACT, straight from PSUM; max-shift skipped —
                                logits are O(+-10) so fp32 exp is safe)
  G[m,:], s[m] = p @ [x | 1]   (PE, via PE-transposed p tiles)
  out = (G/s) @ W_l + bias     (tiny [128,128] tail)

n is processed in two column-halves; half 0's exp/transpose/aggregate tail is
folded into half 1's producer-bound PE stream. The per-seed channel reduce
uses 32-column zero-padded stationaries cycling the four PE column-groups
(consecutive matmuls hit different col_grps and overlap ~2.6x). Seed-side
prep (srT = seed@W_r, fused 0.2*W_l@A, Q, masks) is host weight prep.
"""
import os
import sys
from contextlib import ExitStack

import numpy as np

for _p in ("/root/.axon_site", "/root/.axon_site/_ro/trn_rl_repo",
           "/root/.axon_site/_ro/pypackages", "/opt/trn_rl_repo"):
    if os.path.isdir(_p) and _p not in sys.path:
        sys.path.append(_p)

import concourse.bass as bass
import concourse.bacc as bacc
import concourse.tile as tile
import concourse.mybir as mybir
from concourse.bass_utils import run_bass_kernel_spmd

F32 = mybir.dt.float32
BF16 = mybir.dt.bfloat16
AF = mybir.ActivationFunctionType
ALU = mybir.AluOpType
NP_BF16 = mybir.dt.np(BF16)

B, F, RATIO, H, C = 8, 128, 32, 4, 32
N_CAP = 2304


def _halves_of(n_cap):
    # sections of <=1024 columns: the per-section logit PSUM tile is exactly
    # <=2 banks, letting two sections double-buffer within the 8-bank budget
    out = []
    o = 0
    while o < n_cap:
        sz = min(1024, n_cap - o)
        out.append((o, sz))
        o += sz
    return out

# traversal order interleaves the four 32-row PSUM column-groups (r//8) so
# consecutive PE matmuls target different col_grps of the array
R_ORDER = [8 * (i % 4) + i // 4 for i in range(RATIO)]
# relu-stream engine by traversal position so ACT/DVE work interleaves
# (GPSIMD tensor ops measured 17us/op + SBUF port contention — never use)
ENGINE_OF_POS = ["A" if i % 3 == 1 else "V" for i in range(RATIO)]
# seeds handled by DVE use the one-op form max(h, -s) = relu(h+s) - s; the
# missing +0.8*A^T s term is folded into the host Q correction per row
DVE_SEEDS = {r for i, r in enumerate(R_ORDER) if ENGINE_OF_POS[i] == "V"}

_CACHE = {}


def _chunks_of(off0, size):
    out = []
    o = 0
    while o < size:
        sz = min(512, size - o)
        out.append((off0 + o, sz))
        o += sz
    return out


def build_bass(n_cap=N_CAP):
    nc = bacc.Bacc("TRN2", target_bir_lowering=False, debug=False)
    nt = n_cap // 128
    xT = nc.declare_dram_parameter("xT", [128, n_cap], BF16, isOutput=False)
    xn = nc.declare_dram_parameter("xn", [128, nt * 129], BF16, isOutput=False)
    wl = nc.declare_dram_parameter("W_l", [128, 128], BF16, isOutput=False)
    was = nc.declare_dram_parameter("WAS", [128, 128], BF16, isOutput=False)
    srT = nc.declare_dram_parameter("srTn", [128, 64], F32, isOutput=False)
    abig = nc.declare_dram_parameter("A_big", [128, RATIO * 32], BF16,
                                     isOutput=False)
    selqm = nc.declare_dram_parameter("SelQm", [2, 128], BF16, isOutput=False)
    mask = nc.declare_dram_parameter("maskones", [2, n_cap], BF16, isOutput=False)
    bias1 = nc.declare_dram_parameter("bias1", [1, 128], F32, isOutput=False)
    qcol = nc.declare_dram_parameter("qcol", [128, 1], F32, isOutput=False)
    ident = nc.declare_dram_parameter("ident", [128, 128], BF16, isOutput=False)
    outp = nc.declare_dram_parameter("out", [128, 128], F32, isOutput=True)

    halves = _halves_of(n_cap)

    with ExitStack() as ctx:
        tc = ctx.enter_context(tile.TileContext(nc))
        consts = ctx.enter_context(tc.tile_pool(name="consts", bufs=1))
        big = ctx.enter_context(tc.tile_pool(name="big", bufs=1))
        rpool = ctx.enter_context(tc.tile_pool(name="rpool", bufs=12))
        ps_tmp = ctx.enter_context(tc.tile_pool(name="ps_tmp", bufs=2, space="PSUM"))
        ps_lg = ctx.enter_context(tc.tile_pool(name="ps_lg", bufs=2, space="PSUM"))
        ps_ptr = ctx.enter_context(tc.tile_pool(name="ps_ptr", bufs=1, space="PSUM"))
        ps_os = ctx.enter_context(tc.tile_pool(name="ps_os", bufs=1, space="PSUM"))

        # ---- inputs in, spread across the sync/gpsimd DMA queues (keep the
        #      ACT queue free for the relu stream) ----
        wl_sb = consts.tile([128, 128], BF16, tag="wl")
        nc.sync.dma_start(out=wl_sb, in_=wl[:])
        xT_sb = big.tile([128, n_cap], BF16, tag="xT")
        was_sb = consts.tile([128, 128], BF16, tag="was")
        nc.scalar.dma_start(out=was_sb, in_=was[:])
        xq = [0] + [(n_cap * i // 512) * 128 for i in (1, 2, 3)] + [n_cap]
        for qi, (a, b) in enumerate(zip(xq, xq[1:])):
            eng = nc.sync if qi % 2 == 0 else nc.scalar
            eng.dma_start(out=xT_sb[:, a:b], in_=xT[:, a:b])
        xn_sb = big.tile([128, nt * 129], BF16, tag="xn")
        nc.scalar.dma_start(out=xn_sb, in_=xn[:])
        abig_sb = consts.tile([128, RATIO * 32], BF16, tag="abig")
        nc.sync.dma_start(out=abig_sb, in_=abig[:])
        ident_sb = consts.tile([128, 128], BF16, tag="ident")
        nc.sync.dma_start(out=ident_sb, in_=ident[:])
        selqm_sb = consts.tile([2, 128], BF16, tag="selqm")
        nc.gpsimd.dma_start(out=selqm_sb, in_=selqm[:])
        mask_sb = consts.tile([2, n_cap], BF16, tag="mask")
        nc.gpsimd.dma_start(out=mask_sb, in_=mask[:])
        srT_sb = consts.tile([128, 64], F32, tag="srt")
        nc.gpsimd.dma_start(out=srT_sb, in_=srT[:])
        qcol_sb = consts.tile([128, 1], F32, tag="qcol")
        nc.gpsimd.dma_start(out=qcol_sb, in_=qcol[:])
        biasb_sb = consts.tile([128, 128], F32, tag="biasb")
        b1 = bias1[0:1, :]
        nc.gpsimd.dma_start(
            out=biasb_sb,
            in_=bass.AP(tensor=b1.tensor, offset=b1.offset, ap=[[0, 128], [1, 128]]),
        )

        # ---- h_srcT = W_l^T @ xT (ACT relus read the full width, so all
        #      chunks are evacuated up front) ----
        hsrcT_sb = big.tile([128, n_cap], BF16, tag="hsrcT")
        for off, sz in _chunks_of(0, n_cap):
            pt = ps_tmp.tile([128, 512], F32, tag="tmp")
            nc.tensor.matmul(pt[:, :sz], lhsT=wl_sb, rhs=xT_sb[:, off:off + sz],
                             start=True, stop=True)
            nc.vector.tensor_copy(hsrcT_sb[:, off:off + sz], pt[:, :sz])
        rapool = ctx.enter_context(tc.tile_pool(name="rapool", bufs=1))
        p_sb = big.tile([128, n_cap], BF16, tag="p")
        pT_sb = big.tile([128, nt * 128], BF16, tag="pT")
        g_ps = ps_os.tile([128, 129], F32, tag="G")

        def emit_exp(h_off, h_sz, lg):
            # softmax numerator straight out of PSUM (no logit evacuation)
            nc.scalar.activation(p_sb[:, h_off:h_off + h_sz],
                                 lg[:, 0:h_sz], AF.Exp, bias=qcol_sb[:, 0:1])

        def emit_transpose(tiles):
            # PE-transpose p tiles (4 per PSUM bank) and evacuate
            for b0 in range(0, len(tiles), 4):
                bt = tiles[b0:b0 + 4]
                ptr = ps_ptr.tile([128, 512], BF16, tag="ptr")
                for j, t in enumerate(bt):
                    nc.tensor.transpose(ptr[:, 128 * j:128 * (j + 1)],
                                        p_sb[:, 128 * t:128 * (t + 1)], ident_sb)
                nc.vector.tensor_copy(
                    pT_sb[:, 128 * bt[0]:128 * (bt[-1] + 1)],
                    ptr[:, :128 * len(bt)])

        def emit_agg(tiles, first, last):
            for t in tiles:
                nc.tensor.matmul(g_ps[:, :],
                                 lhsT=pT_sb[:, 128 * t:128 * (t + 1)],
                                 rhs=xn_sb[:, 129 * t:129 * (t + 1)],
                                 start=(first and t == tiles[0]),
                                 stop=(last and t == tiles[-1]))

        pending = None
        ract = {}
        for hi, (h_off, h_sz) in enumerate(halves):
            chunks = _chunks_of(h_off, h_sz)
            # per-section logit PSUM tile (<=2 banks, double-buffered pool);
            # each matmul stays within a single 2KB bank slice of it
            lg = ps_lg.tile([128, 1024], F32, tag="lg")
            for ci, (off, sz) in enumerate(chunks):
                o = off - h_off
                nc.tensor.matmul(lg[:, o:o + sz], lhsT=was_sb,
                                 rhs=xT_sb[:, off:off + sz],
                                 start=True, stop=False, skip_group_check=True)
                nc.tensor.matmul(lg[:, o:o + sz], lhsT=selqm_sb,
                                 rhs=mask_sb[:, off:off + sz],
                                 start=False, stop=False, skip_group_check=True)

            for pos, r in enumerate(R_ORDER):
                R = rpool.tile([128, h_sz], BF16, tag=f"R{hi}")
                base = h_off
                if ENGINE_OF_POS[pos] == "A":
                    nc.scalar.activation(R, hsrcT_sb[:, h_off:h_off + h_sz],
                                         AF.Relu, bias=srT_sb[:, r:r + 1],
                                         scale=1.0)
                else:
                    nc.vector.tensor_scalar_max(
                        R, hsrcT_sb[:, h_off:h_off + h_sz],
                        srT_sb[:, 32 + r:33 + r])
                g32 = 32 * (r // 8)
                for ci, (off, sz) in enumerate(chunks):
                    o = off - h_off
                    nc.tensor.matmul(lg[g32:g32 + 32, o:o + sz],
                                     lhsT=abig_sb[:, 32 * r:32 * r + 32],
                                     rhs=R[:, off - base:off - base + sz],
                                     start=False, stop=(r % 8 == 7),
                                     tile_position=(0, g32),
                                     skip_group_check=True)
                if pos == 8 and pending is not None:
                    # fold the previous section's transposes into this
                    # section's (producer-bound) PE stream ...
                    emit_transpose(pending[0])
                if pos == 18 and pending is not None:
                    # ... and its aggregation matmuls a bit later, giving the
                    # DVE time to evacuate the transposed tiles
                    emit_agg(*pending)
                    pending = None

            if hi < len(halves) - 1:
                emit_exp(h_off, h_sz, lg)
                pending = (list(range(h_off // 128, (h_off + h_sz) // 128)),
                           hi == 0, False)
            else:
                # final section: pipeline exp -> transpose -> aggregate
                for off, sz in _chunks_of(0, h_sz):
                    nc.scalar.activation(
                        p_sb[:, h_off + off:h_off + off + sz],
                        lg[:, off:off + sz], AF.Exp, bias=qcol_sb[:, 0:1])
                    tiles = list(range((h_off + off) // 128,
                                       (h_off + off + sz) // 128))
                    emit_transpose(tiles)
                    emit_agg(tiles, first=(hi == 0 and off == 0),
                             last=(off + sz == h_sz))

        # ---- tail: out = (G/s) @ W_l + bias ----
        srecip = big.tile([128, 1], F32, tag="srecip")
        nc.vector.reciprocal(srecip, g_ps[:, 128:129])
        gn_sb = big.tile([128, 128], BF16, tag="gn")
        nc.vector.tensor_scalar(gn_sb, g_ps[:, 0:128], scalar1=srecip,
                                scalar2=None, op0=ALU.mult)
        gnT_ps = ps_tmp.tile([128, 128], BF16, tag="tmp")
        nc.tensor.transpose(gnT_ps, gn_sb, ident_sb)
        gnT_sb = big.tile([128, 128], BF16, tag="gnT")
        nc.vector.tensor_copy(gnT_sb, gnT_ps)
        out_ps = ps_tmp.tile([128, 128], F32, tag="tmp")
        nc.tensor.matmul(out_ps, lhsT=gnT_sb, rhs=wl_sb, start=True, stop=True)
        out_sb = big.tile([128, 128], F32, tag="out_sb")
        nc.vector.tensor_add(out_sb, out_ps, biasb_sb)

        # full [m=4r+h, f] result out; host extracts out[r, 32h:32h+32] =
        # out_sb[4r+h, 32h:32h+32] during unshard
        nc.sync.dma_start(out=outp[:], in_=out_sb[:])

    nc.compile()
    return nc


def host_prep(x, batch, seed_nodes, W_l, W_r, att, bias, n_cap=N_CAP):
    f32 = np.float32
    x = np.asarray(x, f32)
    batch = np.asarray(batch).astype(np.int32)
    seed_nodes = np.asarray(seed_nodes, f32)
    W_l = np.asarray(W_l, f32)
    W_r = np.asarray(W_r, f32)
    att = np.asarray(att, f32)
    bias = np.asarray(bias, f32)
    nt = n_cap // 128

    order = np.argsort(batch, kind="stable")
    x_sorted = x[order]
    counts = np.bincount(batch[order], minlength=B)
    offs = np.concatenate([[0], np.cumsum(counts)])

    seed_hr = seed_nodes @ W_r                       # [32,128]
    A = np.zeros((F, H), f32)
    for h in range(H):
        A[h * C:(h + 1) * C, h] = att[h]
    # A_big[:, 32r + 4(r%8) + h] = 0.8*A[:, h]; the per-seed matmul uses the
    # 32-col slice [:, 32r:32r+32] writing PSUM rows [32*(r//8), +32) so
    # row m = 4r+h while output base partitions stay 32-aligned.
    A_big = np.zeros((F, RATIO * 32), f32)
    for r in range(RATIO):
        A_big[:, 32 * r + 4 * (r % 8):32 * r + 4 * (r % 8) + 4] = 0.8 * A
    m = np.arange(128)
    WAS = 0.2 * (W_l @ A)[:, m % 4]                  # [128, 128]
    # per-row constant: 0.2*A^T s_r always, plus 0.8*A^T s_r for seeds whose
    # relu runs as max(h, -s) on DVE (their A-matmul contribution is missing
    # the +0.8*A^T s term)
    AS = np.einsum("rf,fh->hr", seed_hr, A)          # [4,32]
    qscale = np.array([1.0 if (mm // 4) in DVE_SEEDS else 0.2
                       for mm in m], f32)
    qcol = np.ascontiguousarray((AS[m % 4, m // 4] * qscale)[:, None])
    SelQm = np.zeros((2, 128), f32)
    SelQm[1, :] = 1.0

    shared = dict(
        W_l=np.ascontiguousarray(W_l.astype(NP_BF16)),
        WAS=np.ascontiguousarray(WAS.astype(NP_BF16)),
        srTn=np.ascontiguousarray(
            np.concatenate([seed_hr.T, -seed_hr.T], axis=1)),
        A_big=A_big.astype(NP_BF16),
        SelQm=SelQm.astype(NP_BF16),
        bias1=np.ascontiguousarray(bias[None, :]),
        qcol=qcol,
        ident=np.eye(128, dtype=f32).astype(NP_BF16),
    )
    in_maps = []
    for b in range(B):
        n_b = int(counts[b])
        assert n_b <= n_cap, f"graph {b}: {n_b} nodes > N_CAP {n_cap}"
        xb = np.zeros((n_cap, F), f32)
        xb[:n_b] = x_sorted[offs[b]:offs[b + 1]]
        # xn: [x | 1] tiled into the SBUF layout [128, nt*129]
        x1 = np.concatenate([xb, np.ones((n_cap, 1), f32)], axis=1)
        xn = np.ascontiguousarray(
            x1.reshape(nt, 128, 129).transpose(1, 0, 2).reshape(128, nt * 129))
        maskones = np.zeros((2, n_cap), f32)
        maskones[0, :] = 1.0            # multiplies SelQm row 0 (Q correction)
        maskones[1, n_b:] = -50.0       # multiplies SelQm row 1 (ones)
        in_maps.append(dict(
            shared,
            xT=np.ascontiguousarray(xb.T.astype(NP_BF16)),
            xn=xn.astype(NP_BF16),
            maskones=maskones.astype(NP_BF16),
        ))
    return in_maps


def kernel(x, batch, seed_nodes, W_l, W_r, att, bias):
    counts = np.bincount(np.asarray(batch).astype(np.int64), minlength=B)
    n_cap = max(2048, int(-(-int(counts.max()) // 128) * 128))
    if n_cap not in _CACHE:
        _CACHE[n_cap] = build_bass(n_cap)
    nc = _CACHE[n_cap]
    in_maps = host_prep(x, batch, seed_nodes, W_l, W_r, att, bias, n_cap=n_cap)
    res = run_bass_kernel_spmd(nc, in_maps, core_ids=list(range(B)))
    out = np.concatenate([unshard_core(np.asarray(res.results[i]["out"]))
                          for i in range(B)], axis=0)
    new_batch = np.repeat(np.arange(B, dtype=np.int32), RATIO)
    return out, new_batch


def unshard_core(out128):
    out = np.empty((RATIO, F), np.float32)
    for h in range(H):
        out[:, 32 * h:32 * (h + 1)] = out128[h::4, 32 * h:32 * (h + 1)]
    return out


# revision 65
# speedup vs baseline: 1.1463x; 1.0435x over previous
"""Trainium2 Bass kernel for nn_BipartitePooling (GATv2 bipartite pooling).

Sharding: one graph per NeuronCore (8 graphs, 8 cores, no collectives).
Per core, for its graph's nodes (padded to N_CAP columns, pad columns pushed
to -inf logits via a mask row):

  h_srcT[f,n] = W_l^T @ xT                                   (PE)
  logit[4r+h, n] = 0.8*A^T relu(h_srcT + srT[:,r])           (relu stream on
      + (0.2*W_l@A)^T @ xT + Q[r,h] + maskbias[n]             ACT+DVE, reduce
                                                              on PE into PSUM)
  p = exp(logit)               (ACT, straight from PSUM; max-shift skipped —
                                logits are O(+-10) so fp32 exp is safe)
  G[m,:], s[m] = p @ [x | 1]   (PE, via PE-transposed p tiles)
  out = (G/s) @ W_l + bias     (tiny [128,128] tail)

n is processed in two column-halves; half 0's exp/transpose/aggregate tail is
folded into half 1's producer-bound PE stream. The per-seed channel reduce
uses 32-column zero-padded stationaries cycling the four PE column-groups
(consecutive matmuls hit different col_grps and overlap ~2.6x). Seed-side
prep (srT = seed@W_r, fused 0.2*W_l@A, Q, masks) is host weight prep.
"""
import os
import sys
from contextlib import ExitStack

import numpy as np

for _p in ("/root/.axon_site", "/root/.axon_site/_ro/trn_rl_repo",
           "/root/.axon_site/_ro/pypackages", "/opt/trn_rl_repo"):
    if os.path.isdir(_p) and _p not in sys.path:
        sys.path.append(_p)

import concourse.bass as bass
import concourse.bacc as bacc
import concourse.tile as tile
import concourse.mybir as mybir
from concourse.bass_utils import run_bass_kernel_spmd

F32 = mybir.dt.float32
BF16 = mybir.dt.bfloat16
AF = mybir.ActivationFunctionType
ALU = mybir.AluOpType
NP_BF16 = mybir.dt.np(BF16)

B, F, RATIO, H, C = 8, 128, 32, 4, 32
N_CAP = 2304


def _halves_of(n_cap):
    # sections of <=1024 columns: the per-section logit PSUM tile is exactly
    # <=2 banks, letting two sections double-buffer within the 8-bank budget
    out = []
    o = 0
    while o < n_cap:
        sz = min(1024, n_cap - o)
        out.append((o, sz))
        o += sz
    return out

# traversal order interleaves the four 32-row PSUM column-groups (r//8) so
# consecutive PE matmuls target different col_grps of the array
R_ORDER = [8 * (i % 4) + i // 4 for i in range(RATIO)]
# relu-stream engine by traversal position so ACT/DVE work interleaves
# (GPSIMD tensor ops measured 17us/op + SBUF port contention — never use)
ENGINE_OF_POS = ["A" if (i % 3 == 1 and i != 28) else "V" for i in range(RATIO)]
# seeds handled by DVE use the one-op form max(h, -s) = relu(h+s) - s; the
# missing +0.8*A^T s term is folded into the host Q correction per row
DVE_SEEDS = {r for i, r in enumerate(R_ORDER) if ENGINE_OF_POS[i] == "V"}

_CACHE = {}


def _chunks_of(off0, size):
    out = []
    o = 0
    while o < size:
        sz = min(512, size - o)
        out.append((off0 + o, sz))
        o += sz
    return out


def build_bass(n_cap=N_CAP):
    nc = bacc.Bacc("TRN2", target_bir_lowering=False, debug=False)
    nt = n_cap // 128
    xT = nc.declare_dram_parameter("xT", [128, n_cap], BF16, isOutput=False)
    xn = nc.declare_dram_parameter("xn", [128, nt * 129], BF16, isOutput=False)
    wl = nc.declare_dram_parameter("W_l", [128, 128], BF16, isOutput=False)
    was = nc.declare_dram_parameter("WAS", [128, 128], BF16, isOutput=False)
    srT = nc.declare_dram_parameter("srTn", [128, 64], F32, isOutput=False)
    abig = nc.declare_dram_parameter("A_big", [128, RATIO * 32], BF16,
                                     isOutput=False)
    selqm = nc.declare_dram_parameter("SelQm", [2, 128], BF16, isOutput=False)
    mask = nc.declare_dram_parameter("maskones", [2, n_cap], BF16, isOutput=False)
    bias1 = nc.declare_dram_parameter("bias1", [1, 128], F32, isOutput=False)
    qcol = nc.declare_dram_parameter("qcol", [128, 1], F32, isOutput=False)
    ident = nc.declare_dram_parameter("ident", [128, 128], BF16, isOutput=False)
    outp = nc.declare_dram_parameter("out", [128, 128], F32, isOutput=True)

    halves = _halves_of(n_cap)

    with ExitStack() as ctx:
        tc = ctx.enter_context(tile.TileContext(nc))
        consts = ctx.enter_context(tc.tile_pool(name="consts", bufs=1))
        big = ctx.enter_context(tc.tile_pool(name="big", bufs=1))
        rpool = ctx.enter_context(tc.tile_pool(name="rpool", bufs=12))
        ps_tmp = ctx.enter_context(tc.tile_pool(name="ps_tmp", bufs=2, space="PSUM"))
        ps_lg = ctx.enter_context(tc.tile_pool(name="ps_lg", bufs=2, space="PSUM"))
        ps_ptr = ctx.enter_context(tc.tile_pool(name="ps_ptr", bufs=1, space="PSUM"))
        ps_os = ctx.enter_context(tc.tile_pool(name="ps_os", bufs=1, space="PSUM"))

        # ---- inputs in, spread across the sync/gpsimd DMA queues (keep the
        #      ACT queue free for the relu stream) ----
        wl_sb = consts.tile([128, 128], BF16, tag="wl")
        nc.sync.dma_start(out=wl_sb, in_=wl[:])
        xT_sb = big.tile([128, n_cap], BF16, tag="xT")
        was_sb = consts.tile([128, 128], BF16, tag="was")
        nc.scalar.dma_start(out=was_sb, in_=was[:])
        xq = [0] + [(n_cap * i // 512) * 128 for i in (1, 2, 3)] + [n_cap]
        for qi, (a, b) in enumerate(zip(xq, xq[1:])):
            eng = nc.sync if qi % 2 == 0 else nc.scalar
            eng.dma_start(out=xT_sb[:, a:b], in_=xT[:, a:b])
        xn_sb = big.tile([128, nt * 129], BF16, tag="xn")
        nc.scalar.dma_start(out=xn_sb, in_=xn[:])
        abig_sb = consts.tile([128, RATIO * 32], BF16, tag="abig")
        nc.sync.dma_start(out=abig_sb, in_=abig[:])
        ident_sb = consts.tile([128, 128], BF16, tag="ident")
        nc.sync.dma_start(out=ident_sb, in_=ident[:])
        selqm_sb = consts.tile([2, 128], BF16, tag="selqm")
        nc.gpsimd.dma_start(out=selqm_sb, in_=selqm[:])
        mask_sb = consts.tile([2, n_cap], BF16, tag="mask")
        nc.gpsimd.dma_start(out=mask_sb, in_=mask[:])
        srT_sb = consts.tile([128, 64], F32, tag="srt")
        nc.gpsimd.dma_start(out=srT_sb, in_=srT[:])
        qcol_sb = consts.tile([128, 1], F32, tag="qcol")
        nc.gpsimd.dma_start(out=qcol_sb, in_=qcol[:])
        biasb_sb = consts.tile([128, 128], F32, tag="biasb")
        b1 = bias1[0:1, :]
        nc.gpsimd.dma_start(
            out=biasb_sb,
            in_=bass.AP(tensor=b1.tensor, offset=b1.offset, ap=[[0, 128], [1, 128]]),
        )

        # ---- h_srcT = W_l^T @ xT; section 0's chunks up front, later
        #      sections' chunks deferred into the preceding section's stream
        #      so their casts don't block the first relus on the DVE queue ----
        hsrcT_sb = big.tile([128, n_cap], BF16, tag="hsrcT")

        def emit_hsT(sec):
            for off, sz in _chunks_of(*sec):
                pt = ps_tmp.tile([128, 512], F32, tag="tmp")
                nc.tensor.matmul(pt[:, :sz], lhsT=wl_sb,
                                 rhs=xT_sb[:, off:off + sz],
                                 start=True, stop=True)
                nc.vector.tensor_copy(hsrcT_sb[:, off:off + sz], pt[:, :sz])

        emit_hsT(halves[0])
        p_sb = big.tile([128, n_cap], BF16, tag="p")
        pT_sb = big.tile([128, nt * 128], BF16, tag="pT")
        g_ps = ps_os.tile([128, 129], F32, tag="G")

        def emit_exp(h_off, h_sz, lg):
            # softmax numerator straight out of PSUM (no logit evacuation)
            nc.scalar.activation(p_sb[:, h_off:h_off + h_sz],
                                 lg[:, 0:h_sz], AF.Exp, bias=qcol_sb[:, 0:1])

        def emit_transpose(tiles):
            # PE-transpose p tiles (4 per PSUM bank) and evacuate
            for b0 in range(0, len(tiles), 4):
                bt = tiles[b0:b0 + 4]
                ptr = ps_ptr.tile([128, 512], BF16, tag="ptr")
                for j, t in enumerate(bt):
                    nc.tensor.transpose(ptr[:, 128 * j:128 * (j + 1)],
                                        p_sb[:, 128 * t:128 * (t + 1)], ident_sb)
                nc.vector.tensor_copy(
                    pT_sb[:, 128 * bt[0]:128 * (bt[-1] + 1)],
                    ptr[:, :128 * len(bt)])

        def emit_agg(tiles, first, last):
            for t in tiles:
                nc.tensor.matmul(g_ps[:, :],
                                 lhsT=pT_sb[:, 128 * t:128 * (t + 1)],
                                 rhs=xn_sb[:, 129 * t:129 * (t + 1)],
                                 start=(first and t == tiles[0]),
                                 stop=(last and t == tiles[-1]))

        pending = None
        for hi, (h_off, h_sz) in enumerate(halves):
            chunks = _chunks_of(h_off, h_sz)
            # per-section logit PSUM tile (<=2 banks, double-buffered pool);
            # each matmul stays within a single 2KB bank slice of it
            lg = ps_lg.tile([128, 1024], F32, tag="lg")
            for ci, (off, sz) in enumerate(chunks):
                o = off - h_off
                nc.tensor.matmul(lg[:, o:o + sz], lhsT=was_sb,
                                 rhs=xT_sb[:, off:off + sz],
                                 start=True, stop=False, skip_group_check=True)
                nc.tensor.matmul(lg[:, o:o + sz], lhsT=selqm_sb,
                                 rhs=mask_sb[:, off:off + sz],
                                 start=False, stop=False, skip_group_check=True)

            for pos, r in enumerate(R_ORDER):
                R = rpool.tile([128, h_sz], BF16, tag=f"R{hi}")
                base = h_off
                if ENGINE_OF_POS[pos] == "A":
                    nc.scalar.activation(R, hsrcT_sb[:, h_off:h_off + h_sz],
                                         AF.Relu, bias=srT_sb[:, r:r + 1],
                                         scale=1.0)
                else:
                    nc.vector.tensor_scalar_max(
                        R, hsrcT_sb[:, h_off:h_off + h_sz],
                        srT_sb[:, 32 + r:33 + r])
                g32 = 32 * (r // 8)
                for ci, (off, sz) in enumerate(chunks):
                    o = off - h_off
                    nc.tensor.matmul(lg[g32:g32 + 32, o:o + sz],
                                     lhsT=abig_sb[:, 32 * r:32 * r + 32],
                                     rhs=R[:, off - base:off - base + sz],
                                     start=False, stop=(r % 8 == 7),
                                     tile_position=(0, g32),
                                     skip_group_check=True)
                if pos == 4 and hi + 1 < len(halves):
                    emit_hsT(halves[hi + 1])
                if pos == 8 and pending is not None:
                    # fold the previous section's transposes into this
                    # section's (producer-bound) PE stream ...
                    emit_transpose(pending[0])
                if pos == 18 and pending is not None:
                    # ... and its aggregation matmuls a bit later, giving the
                    # DVE time to evacuate the transposed tiles
                    emit_agg(*pending)
                    pending = None

            if hi < len(halves) - 1:
                emit_exp(h_off, h_sz, lg)
                pending = (list(range(h_off // 128, (h_off + h_sz) // 128)),
                           hi == 0, False)
            else:
                # final section: pipeline exp -> transpose -> aggregate
                for off, sz in _chunks_of(0, h_sz):
                    nc.scalar.activation(
                        p_sb[:, h_off + off:h_off + off + sz],
                        lg[:, off:off + sz], AF.Exp, bias=qcol_sb[:, 0:1])
                    tiles = list(range((h_off + off) // 128,
                                       (h_off + off + sz) // 128))
                    emit_transpose(tiles)
                    emit_agg(tiles, first=(hi == 0 and off == 0),
                             last=(off + sz == h_sz))

        # ---- tail: out = (G/s) @ W_l + bias ----
        srecip = big.tile([128, 1], F32, tag="srecip")
        nc.vector.reciprocal(srecip, g_ps[:, 128:129])
        gn_sb = big.tile([128, 128], BF16, tag="gn")
        nc.vector.tensor_scalar(gn_sb, g_ps[:, 0:128], scalar1=srecip,
                                scalar2=None, op0=ALU.mult)
        gnT_ps = ps_tmp.tile([128, 128], BF16, tag="tmp")
        nc.tensor.transpose(gnT_ps, gn_sb, ident_sb)
        gnT_sb = big.tile([128, 128], BF16, tag="gnT")
        nc.vector.tensor_copy(gnT_sb, gnT_ps)
        out_ps = ps_tmp.tile([128, 128], F32, tag="tmp")
        nc.tensor.matmul(out_ps, lhsT=gnT_sb, rhs=wl_sb, start=True, stop=True)
        out_sb = big.tile([128, 128], F32, tag="out_sb")
        nc.vector.tensor_add(out_sb, out_ps, biasb_sb)

        # full [m=4r+h, f] result out; host extracts out[r, 32h:32h+32] =
        # out_sb[4r+h, 32h:32h+32] during unshard
        nc.sync.dma_start(out=outp[:], in_=out_sb[:])

    nc.compile()
    return nc


def host_prep(x, batch, seed_nodes, W_l, W_r, att, bias, n_cap=N_CAP):
    f32 = np.float32
    x = np.asarray(x, f32)
    batch = np.asarray(batch).astype(np.int32)
    seed_nodes = np.asarray(seed_nodes, f32)
    W_l = np.asarray(W_l, f32)
    W_r = np.asarray(W_r, f32)
    att = np.asarray(att, f32)
    bias = np.asarray(bias, f32)
    nt = n_cap // 128

    order = np.argsort(batch, kind="stable")
    x_sorted = x[order]
    counts = np.bincount(batch[order], minlength=B)
    offs = np.concatenate([[0], np.cumsum(counts)])

    seed_hr = seed_nodes @ W_r                       # [32,128]
    A = np.zeros((F, H), f32)
    for h in range(H):
        A[h * C:(h + 1) * C, h] = att[h]
    # A_big[:, 32r + 4(r%8) + h] = 0.8*A[:, h]; the per-seed matmul uses the
    # 32-col slice [:, 32r:32r+32] writing PSUM rows [32*(r//8), +32) so
    # row m = 4r+h while output base partitions stay 32-aligned.
    A_big = np.zeros((F, RATIO * 32), f32)
    for r in range(RATIO):
        A_big[:, 32 * r + 4 * (r % 8):32 * r + 4 * (r % 8) + 4] = 0.8 * A
    m = np.arange(128)
    WAS = 0.2 * (W_l @ A)[:, m % 4]                  # [128, 128]
    # per-row constant: 0.2*A^T s_r always, plus 0.8*A^T s_r for seeds whose
    # relu runs as max(h, -s) on DVE (their A-matmul contribution is missing
    # the +0.8*A^T s term)
    AS = np.einsum("rf,fh->hr", seed_hr, A)          # [4,32]
    qscale = np.array([1.0 if (mm // 4) in DVE_SEEDS else 0.2
                       for mm in m], f32)
    qcol = np.ascontiguousarray((AS[m % 4, m // 4] * qscale)[:, None])
    SelQm = np.zeros((2, 128), f32)
    SelQm[1, :] = 1.0

    shared = dict(
        W_l=np.ascontiguousarray(W_l.astype(NP_BF16)),
        WAS=np.ascontiguousarray(WAS.astype(NP_BF16)),
        srTn=np.ascontiguousarray(
            np.concatenate([seed_hr.T, -seed_hr.T], axis=1)),
        A_big=A_big.astype(NP_BF16),
        SelQm=SelQm.astype(NP_BF16),
        bias1=np.ascontiguousarray(bias[None, :]),
        qcol=qcol,
        ident=np.eye(128, dtype=f32).astype(NP_BF16),
    )
    in_maps = []
    for b in range(B):
        n_b = int(counts[b])
        assert n_b <= n_cap, f"graph {b}: {n_b} nodes > N_CAP {n_cap}"
        xb = np.zeros((n_cap, F), f32)
        xb[:n_b] = x_sorted[offs[b]:offs[b + 1]]
        # xn: [x | 1] tiled into the SBUF layout [128, nt*129]
        x1 = np.concatenate([xb, np.ones((n_cap, 1), f32)], axis=1)
        xn = np.ascontiguousarray(
            x1.reshape(nt, 128, 129).transpose(1, 0, 2).reshape(128, nt * 129))
        maskones = np.zeros((2, n_cap), f32)
        maskones[0, :] = 1.0            # multiplies SelQm row 0 (Q correction)
        maskones[1, n_b:] = -50.0       # multiplies SelQm row 1 (ones)
        in_maps.append(dict(
            shared,
            xT=np.ascontiguousarray(xb.T.astype(NP_BF16)),
            xn=xn.astype(NP_BF16),
            maskones=maskones.astype(NP_BF16),
        ))
    return in_maps


def kernel(x, batch, seed_nodes, W_l, W_r, att, bias):
    counts = np.bincount(np.asarray(batch).astype(np.int64), minlength=B)
    n_cap = max(2048, int(-(-int(counts.max()) // 128) * 128))
    if n_cap not in _CACHE:
        _CACHE[n_cap] = build_bass(n_cap)
    nc = _CACHE[n_cap]
    in_maps = host_prep(x, batch, seed_nodes, W_l, W_r, att, bias, n_cap=n_cap)
    res = run_bass_kernel_spmd(nc, in_maps, core_ids=list(range(B)))
    out = np.concatenate([unshard_core(np.asarray(res.results[i]["out"]))
                          for i in range(B)], axis=0)
    new_batch = np.repeat(np.arange(B, dtype=np.int32), RATIO)
    return out, new_batch


def unshard_core(out128):
    out = np.empty((RATIO, F), np.float32)
    for h in range(H):
        out[:, 32 * h:32 * (h + 1)] = out128[h::4, 32 * h:32 * (h + 1)]
    return out
